# revision 39
# baseline (speedup 1.0000x reference)
"""ABMIL gated-attention MIL pooling on 8 TRN2 NeuronCores.

Work-item data parallelism: every 512-token group of every bag is an
independent work item; the ceil(G_tot/8) items per core are balanced
across cores (vs. bag-parallel, where every SPMD core pays for the
longest bag).  Per item (512 tokens, D=1024, H=256):

    A   = tanh(x Vw + Vb) * sigmoid(x Uw + Ub)        [512, H]
    s   = A Ww                                        [512]
    e   = exp(s) * mask                               [512]   (no max-sub:
          |s| <= sum|0.5 W| ~ 13, exp fits f32/bf16 easily)
    zk  = e @ x_group,  dk = sum(e)                   [D], [1]

Host combines: Z_b = (sum_k zk) / (sum_k dk) over the bag's items.
Wb shifts every score equally -> cancels -> dropped.

Per-core pipeline (bf16 compute / f32 accumulate):
  - x^T group [128 d, 8 dc, 512 tok] bf16, host-pretransposed, one load
    (pooling runs from the same layout -> half the HBM traffic)
  - projections on TensorE (contract d); tanh on ScalarE with
    sigmoid(z) = 0.5*tanh(z/2)+0.5 folded as A.W = (0.5W).(tv*(tu+1))
  - gate (tu+1)*tv fused in one VectorE scalar_tensor_tensor
  - scores: 2 accumulating [128,1]x[128,512] matmuls
  - exp on ScalarE; mask*exp + denom in one VectorE tensor_tensor_reduce
  - e broadcast to 128 partitions on GpSimd; pooling = 8 VectorE
    tensor_tensor_reduce ops (xT[:,dc,:]*e -> accum z[:,k,dc])
"""

import math
import os

import numpy as np
import ml_dtypes

import concourse.bass as bass
import concourse.bacc as bacc
import concourse.tile as tile
from concourse import mybir, bass_isa
from concourse.bass_utils import run_bass_kernel_spmd

F32 = mybir.dt.float32
BF16 = mybir.dt.bfloat16
F8 = mybir.dt.float8e4
NPF8 = mybir.dt.np(F8)
DR = mybir.MatmulPerfMode.DoubleRow
AF = mybir.ActivationFunctionType
OP = mybir.AluOpType

STAGE = int(os.environ.get("KSTAGE", "3"))  # HW bisect: 0=proj,1=+scores/exp,2=+bcast,3=full

B, N, D, H = 16, 4096, 1024, 256
NCORES = 8
P = 128                    # partitions
NTOK = 512                 # tokens per work item
NG = N // NTOK             # max items per bag = 8
DC = D // P                # 8 d-chunks
HC = H // P                # 2 h-chunks


def build_graph(K):
    nc = bacc.Bacc(None)
    xt_ext = nc.declare_dram_parameter("xT", [K, P, DC, NTOK], BF16, isOutput=False)
    xt8_ext = nc.declare_dram_parameter("xT8", [K, P, DC, NTOK], F8, isOutput=False)
    vw_ext = nc.declare_dram_parameter("Vw", [P, DC, HC, P], F8, isOutput=False)
    uw_ext = nc.declare_dram_parameter("Uw", [P, DC, HC, P], F8, isOutput=False)
    vb_ext = nc.declare_dram_parameter("Vb", [P, HC], F32, isOutput=False)
    ubh_ext = nc.declare_dram_parameter("Ubh", [P, HC], F32, isOutput=False)
    w2_ext = nc.declare_dram_parameter("W2", [P, HC], BF16, isOutput=False)
    mask_ext = nc.declare_dram_parameter("mask", [K, 1, NTOK], BF16, isOutput=False)
    outz_ext = nc.declare_dram_parameter("out_z", [P, K, DC], F32, isOutput=True)
    outd_ext = nc.declare_dram_parameter("out_den", [1, K], F32, isOutput=True)
    with tile.TileContext(nc) as tc:
        with (
            tc.tile_pool(name="xt", bufs=4) as p_xt,
            tc.tile_pool(name="x8", bufs=4) as p_x8,
            tc.tile_pool(name="act", bufs=3) as p_act,
            tc.tile_pool(name="small", bufs=3) as p_small,
            tc.tile_pool(name="scr", bufs=2) as p_scr,
            tc.tile_pool(name="one", bufs=1) as p_one,
            tc.tile_pool(name="pproj", bufs=4, space="PSUM") as p_proj,
            tc.tile_pool(name="psml", bufs=2, space="PSUM") as p_ps,
            tc.tile_pool(name="pbc", bufs=2, space="PSUM") as p_bc,
        ):
            v_sb = p_one.tile([P, DC, HC, P], F8, tag="vw")
            u_sb = p_one.tile([P, DC, HC, P], F8, tag="uw")
            for h in range(4):
                sl = slice(h * DC // 4, (h + 1) * DC // 4)
                nc.scalar.dma_start(out=v_sb[:, sl], in_=vw_ext[:, sl])
                nc.scalar.dma_start(out=u_sb[:, sl], in_=uw_ext[:, sl])
            vb_sb = p_one.tile([P, HC], F32, tag="vb")
            ubh_sb = p_one.tile([P, HC], F32, tag="ubh")
            nc.scalar.dma_start(out=vb_sb, in_=vb_ext[:, :])
            nc.scalar.dma_start(out=ubh_sb, in_=ubh_ext[:, :])
            w2_sb = p_one.tile([P, HC], BF16, tag="w2")
            nc.scalar.dma_start(out=w2_sb, in_=w2_ext[:, :])
            ones_sb = p_one.tile([1, P], BF16, tag="ones")
            nc.vector.memset(ones_sb, 1.0)
            zero_sb = p_one.tile([1, 1], F32, tag="zero")
            nc.vector.memset(zero_sb, 0.0)

            den_sb = p_one.tile([1, K], F32, tag="den")
            z_sb = p_one.tile([P, K, DC], F32, tag="z")
            nc.vector.memset(den_sb, 1.0)
            nc.vector.memset(z_sb, 0.0)

            for k in range(K):
                xt8 = p_x8.tile([P, DC, NTOK], F8, tag="xt8", name=f"xt8_{k}")
                for h in range(4):
                    sl = slice(h * DC // 4, (h + 1) * DC // 4)
                    nc.sync.dma_start(out=xt8[:, sl], in_=xt8_ext[k, :, sl])
                xt = p_xt.tile([P, DC, NTOK], BF16, tag="xt", name=f"xt{k}")
                for h in range(2):
                    sl = slice(h * DC // 2, (h + 1) * DC // 2)
                    nc.sync.dma_start(out=xt[:, sl], in_=xt_ext[k, :, sl])
                mk = p_small.tile([1, NTOK], BF16, tag="mk", name=f"mk{k}")
                nc.gpsimd.dma_start(out=mk, in_=mask_ext[k])

                tv = p_act.tile([P, HC, NTOK], BF16, tag="tv", name=f"tv{k}")
                tu = p_act.tile([P, HC, NTOK], BF16, tag="tu", name=f"tu{k}")
                for hc in range(HC):
                    psv = p_proj.tile([P, NTOK], F32, tag="proj", name=f"psv{k}_{hc}")
                    psu = p_proj.tile([P, NTOK], F32, tag="proj", name=f"psu{k}_{hc}")
                    for j in range(DC // 2):
                        d2 = slice(2 * j, 2 * j + 2)
                        nc.tensor.matmul(psv, v_sb[:, d2, hc, :], xt8[:, d2, :],
                                         start=(j == 0), stop=(j == DC // 2 - 1),
                                         perf_mode=DR)
                    for j in range(DC // 2):
                        d2 = slice(2 * j, 2 * j + 2)
                        nc.tensor.matmul(psu, u_sb[:, d2, hc, :], xt8[:, d2, :],
                                         start=(j == 0), stop=(j == DC // 2 - 1),
                                         perf_mode=DR)
                    # weights pre-scaled by 32 on host for fp8 range
                    nc.scalar.activation(out=tv[:, hc, :], in_=psv, func=AF.Tanh,
                                         bias=vb_sb[:, hc:hc + 1], scale=1.0 / 32)
                    nc.scalar.activation(out=tu[:, hc, :], in_=psu, func=AF.Tanh,
                                         bias=ubh_sb[:, hc:hc + 1], scale=0.5 / 32)
                g = p_act.tile([P, HC, NTOK], BF16, tag="g", name=f"g{k}")
                # A.W = (0.5W).(tv*(tu+1)):  g = (tu + 1) * tv
                nc.vector.scalar_tensor_tensor(out=g, in0=tu, scalar=1.0, in1=tv,
                                               op0=OP.add, op1=OP.mult)
                if STAGE < 1:
                    continue
                ps_s = p_ps.tile([1, NTOK], F32, tag="ps", name=f"pss{k}")
                for hc in range(HC):
                    nc.tensor.matmul(ps_s, w2_sb[:, hc:hc + 1], g[:, hc, :],
                                     start=(hc == 0), stop=(hc == HC - 1))
                e_sb = p_small.tile([1, NTOK], BF16, tag="e", name=f"e{k}")
                nc.scalar.activation(out=e_sb, in_=ps_s, func=AF.Exp,
                                     bias=zero_sb[:, :], scale=1.0)
                em = p_small.tile([1, NTOK], BF16, tag="em", name=f"em{k}")
                nc.vector.scalar_tensor_tensor(out=em, in0=e_sb, scalar=1.0,
                                               in1=mk, op0=OP.mult, op1=OP.mult,
                                               accum_out=den_sb[:, k:k + 1])
                if STAGE < 2:
                    continue
                bc_ps = p_bc.tile([P, NTOK], F32, tag="bc", name=f"bc{k}")
                nc.tensor.matmul(bc_ps, ones_sb, em, start=True, stop=True)
                if STAGE < 3:
                    continue
                scr = p_scr.tile([P, NTOK], BF16, tag="scr", name=f"scr{k}")
                for dc in range(DC):
                    nc.vector.scalar_tensor_tensor(
                        out=scr, in0=xt[:, dc, :], scalar=1.0, in1=bc_ps,
                        op0=OP.mult, op1=OP.mult,
                        accum_out=z_sb[:, k, dc:dc + 1])

            nc.sync.dma_start(out=outz_ext[:, :, :], in_=z_sb)
            nc.gpsimd.dma_start(out=outd_ext[:, :], in_=den_sb)

    nc.finalize()
    return nc


_GRAPHS = {}


def _get_graph(K):
    if K not in _GRAPHS:
        _GRAPHS[K] = build_graph(K)
    return _GRAPHS[K]


def _prep_host(x, lengths, V_w, V_b, U_w, U_b, W_w, W_b):
    lengths = np.maximum(np.asarray(lengths).astype(np.int64), 1)
    groups = np.minimum((lengths + NTOK - 1) // NTOK, NG)
    items = [(b, gi) for b in range(B) for gi in range(int(groups[b]))]
    K = math.ceil(len(items) / NCORES)
    assign = [items[c * K:(c + 1) * K] for c in range(NCORES)]

    def warr(w):  # [D, H] -> [dp, dc, hc, h] fp8, pre-scaled by 32
        return np.ascontiguousarray(
            (w * 32.0).reshape(DC, P, HC, P).transpose(1, 0, 2, 3).astype(NPF8))
    Vw = warr(V_w)
    Uw = warr(U_w)
    Vb = np.ascontiguousarray(V_b.reshape(HC, P).T, dtype=np.float32)
    Ubh = np.ascontiguousarray((U_b * 0.5).reshape(HC, P).T, dtype=np.float32)
    W2 = np.ascontiguousarray(
        (0.5 * W_w[:, 0]).reshape(HC, P).T.astype(ml_dtypes.bfloat16))

    xbf = x.astype(ml_dtypes.bfloat16)  # [B, N, D]
    ar = np.arange(NTOK)

    in_maps = []
    for c in range(NCORES):
        xts = np.zeros((K, P, DC, NTOK), dtype=ml_dtypes.bfloat16)
        msk = np.zeros((K, 1, NTOK), dtype=ml_dtypes.bfloat16)
        for k, (b, gi) in enumerate(assign[c]):
            xg = xbf[b, gi * NTOK:(gi + 1) * NTOK, :]        # [512, 1024]
            xts[k] = xg.reshape(NTOK, DC, P).transpose(2, 1, 0)
            msk[k, 0] = (gi * NTOK + ar < lengths[b])
        in_maps.append({"xT": xts, "xT8": xts.astype(NPF8), "mask": msk,
                        "Vw": Vw, "Uw": Uw,
                        "Vb": Vb, "Ubh": Ubh, "W2": W2})
    return in_maps, assign, K


def kernel(x, lengths, V_w, V_b, U_w, U_b, W_w, W_b, _trace=False, _trace_kwargs=None):
    x = np.asarray(x)
    in_maps, assign, K = _prep_host(
        x, lengths, np.asarray(V_w), np.asarray(V_b), np.asarray(U_w),
        np.asarray(U_b), np.asarray(W_w), np.asarray(W_b),
    )
    nc = _get_graph(K)
    res = run_bass_kernel_spmd(
        nc, in_maps, core_ids=list(range(NCORES)),
        trace=_trace, **(_trace_kwargs or {}),
    )
    z = np.zeros((B, D), dtype=np.float64)
    den = np.zeros((B,), dtype=np.float64)
    for c in range(NCORES):
        zc = np.asarray(res.results[c]["out_z"], dtype=np.float64)   # [P, K, DC]
        dc_ = np.asarray(res.results[c]["out_den"], dtype=np.float64)  # [1, K]
        for k, (b, gi) in enumerate(assign[c]):
            z[b] += zc[:, k, :].T.reshape(D)   # d = dc*128 + p
            den[b] += dc_[0, k]
    out = (z / den[:, None]).astype(np.float32)
    if _trace:
        return out, res
    return out


if __name__ == "__main__":
    rng = np.random.default_rng(0)
    x = rng.standard_normal((B, N, D), dtype=np.float32)
    lengths = rng.integers(0, N, (B,)).astype(np.int32)
    s = 1.0 / np.sqrt(D)
    inputs = dict(
        x=x, lengths=lengths,
        V_w=(rng.standard_normal((D, H), dtype=np.float32) * s),
        V_b=np.zeros(H, np.float32),
        U_w=(rng.standard_normal((D, H), dtype=np.float32) * s),
        U_b=np.zeros(H, np.float32),
        W_w=(rng.standard_normal((H, 1), dtype=np.float32) / 16.0),
        W_b=np.zeros(1, np.float32),
    )
    out = kernel(**inputs)
    print(out.shape, out.dtype)


# revision 42
# speedup vs baseline: 1.1324x; 1.1324x over previous
"""ABMIL gated-attention MIL pooling on 8 TRN2 NeuronCores.

Work-item data parallelism: every 512-token group of every bag is an
independent work item; the ceil(G_tot/8) items per core are balanced
across cores (vs. bag-parallel, where every SPMD core pays for the
longest bag).  Per item (512 tokens, D=1024, H=256):

    A   = tanh(x Vw + Vb) * sigmoid(x Uw + Ub)        [512, H]
    s   = A Ww                                        [512]
    e   = exp(s) * mask                               [512]   (no max-sub:
          |s| <= sum|0.5 W| ~ 13, exp fits f32/bf16 easily)
    zk  = e @ x_group,  dk = sum(e)                   [D], [1]

Host combines: Z_b = (sum_k zk) / (sum_k dk) over the bag's items.
Wb shifts every score equally -> cancels -> dropped.

Per-core pipeline (bf16 compute / f32 accumulate):
  - x^T group [128 d, 8 dc, 512 tok] bf16, host-pretransposed, one load
    (pooling runs from the same layout -> half the HBM traffic)
  - projections on TensorE (contract d); tanh on ScalarE with
    sigmoid(z) = 0.5*tanh(z/2)+0.5 folded as A.W = (0.5W).(tv*(tu+1))
  - gate (tu+1)*tv fused in one VectorE scalar_tensor_tensor
  - scores: 2 accumulating [128,1]x[128,512] matmuls
  - exp on ScalarE; mask*exp + denom in one VectorE tensor_tensor_reduce
  - e broadcast to 128 partitions on GpSimd; pooling = 8 VectorE
    tensor_tensor_reduce ops (xT[:,dc,:]*e -> accum z[:,k,dc])
"""

import math
import os

import numpy as np
import ml_dtypes

import concourse.bass as bass
import concourse.bacc as bacc
import concourse.tile as tile
from concourse import mybir, bass_isa
from concourse.bass_utils import run_bass_kernel_spmd

F32 = mybir.dt.float32
BF16 = mybir.dt.bfloat16
F8 = mybir.dt.float8e4
NPF8 = mybir.dt.np(F8)
DR = mybir.MatmulPerfMode.DoubleRow
AF = mybir.ActivationFunctionType
OP = mybir.AluOpType

STAGE = int(os.environ.get("KSTAGE", "3"))  # HW bisect: 0=proj,1=+scores/exp,2=+bcast,3=full

B, N, D, H = 16, 4096, 1024, 256
NCORES = 8
P = 128                    # partitions
NTOK = 512                 # tokens per work item
NG = N // NTOK             # max items per bag = 8
DC = D // P                # 8 d-chunks
HC = H // P                # 2 h-chunks


def build_graph(K):
    nc = bacc.Bacc(None)
    xt_ext = nc.declare_dram_parameter("xT", [K, P, DC, NTOK], BF16, isOutput=False)
    xt8_ext = nc.declare_dram_parameter("xT8", [K, P, DC, NTOK], F8, isOutput=False)
    vw_ext = nc.declare_dram_parameter("Vw", [P, DC, HC, P], F8, isOutput=False)
    uw_ext = nc.declare_dram_parameter("Uw", [P, DC, HC, P], F8, isOutput=False)
    vb_ext = nc.declare_dram_parameter("Vb", [P, HC], F32, isOutput=False)
    ubh_ext = nc.declare_dram_parameter("Ubh", [P, HC], F32, isOutput=False)
    w2_ext = nc.declare_dram_parameter("W2", [P, HC], BF16, isOutput=False)
    mask_ext = nc.declare_dram_parameter("mask", [K, 1, NTOK], BF16, isOutput=False)
    outz_ext = nc.declare_dram_parameter("out_z", [P, K, DC], F32, isOutput=True)
    outd_ext = nc.declare_dram_parameter("out_den", [1, K], F32, isOutput=True)
    with tile.TileContext(nc) as tc:
        with (
            tc.tile_pool(name="xt", bufs=4) as p_xt,
            tc.tile_pool(name="x8", bufs=4) as p_x8,
            tc.tile_pool(name="act", bufs=3) as p_act,
            tc.tile_pool(name="small", bufs=3) as p_small,
            tc.tile_pool(name="scr", bufs=2) as p_scr,
            tc.tile_pool(name="one", bufs=1) as p_one,
            tc.tile_pool(name="pproj", bufs=5, space="PSUM") as p_proj,
            tc.tile_pool(name="psml", bufs=1, space="PSUM") as p_ps,
            tc.tile_pool(name="pbc", bufs=2, space="PSUM") as p_bc,
        ):
            v_sb = p_one.tile([P, DC, HC, P], F8, tag="vw")
            u_sb = p_one.tile([P, DC, HC, P], F8, tag="uw")
            for h in range(2):
                sl = slice(h * DC // 2, (h + 1) * DC // 2)
                nc.scalar.dma_start(out=v_sb[:, sl], in_=vw_ext[:, sl])
                nc.scalar.dma_start(out=u_sb[:, sl], in_=uw_ext[:, sl])
            vb_sb = p_one.tile([P, HC], F32, tag="vb")
            ubh_sb = p_one.tile([P, HC], F32, tag="ubh")
            nc.scalar.dma_start(out=vb_sb, in_=vb_ext[:, :])
            nc.scalar.dma_start(out=ubh_sb, in_=ubh_ext[:, :])
            w2_sb = p_one.tile([P, HC], BF16, tag="w2")
            nc.scalar.dma_start(out=w2_sb, in_=w2_ext[:, :])
            ones_sb = p_one.tile([1, P], BF16, tag="ones")
            nc.vector.memset(ones_sb, 1.0)
            zero_sb = p_one.tile([1, 1], F32, tag="zero")
            nc.vector.memset(zero_sb, 0.0)

            den_sb = p_one.tile([1, K], F32, tag="den")
            z_sb = p_one.tile([P, K, DC], F32, tag="z")
            nc.vector.memset(den_sb, 1.0)
            nc.vector.memset(z_sb, 0.0)

            for k in range(K):
                xt8 = p_x8.tile([P, DC, NTOK], F8, tag="xt8", name=f"xt8_{k}")
                for h in range(2):
                    sl = slice(h * DC // 2, (h + 1) * DC // 2)
                    nc.sync.dma_start(out=xt8[:, sl], in_=xt8_ext[k, :, sl])
                xt = p_xt.tile([P, DC, NTOK], BF16, tag="xt", name=f"xt{k}")
                for h in range(2):
                    sl = slice(h * DC // 2, (h + 1) * DC // 2)
                    nc.sync.dma_start(out=xt[:, sl], in_=xt_ext[k, :, sl])
                mk = p_small.tile([1, NTOK], BF16, tag="mk", name=f"mk{k}")
                nc.gpsimd.dma_start(out=mk, in_=mask_ext[k])

                tv = p_act.tile([P, HC, NTOK], BF16, tag="tv", name=f"tv{k}")
                tu = p_act.tile([P, HC, NTOK], BF16, tag="tu", name=f"tu{k}")
                for hc in range(HC):
                    psv = p_proj.tile([P, NTOK], F32, tag="proj", name=f"psv{k}_{hc}")
                    psu = p_proj.tile([P, NTOK], F32, tag="proj", name=f"psu{k}_{hc}")
                    for j in range(DC // 2):
                        d2 = slice(2 * j, 2 * j + 2)
                        nc.tensor.matmul(psv, v_sb[:, d2, hc, :], xt8[:, d2, :],
                                         start=(j == 0), stop=(j == DC // 2 - 1),
                                         perf_mode=DR)
                    for j in range(DC // 2):
                        d2 = slice(2 * j, 2 * j + 2)
                        nc.tensor.matmul(psu, u_sb[:, d2, hc, :], xt8[:, d2, :],
                                         start=(j == 0), stop=(j == DC // 2 - 1),
                                         perf_mode=DR)
                    # weights pre-scaled by 32 on host for fp8 range
                    nc.scalar.activation(out=tv[:, hc, :], in_=psv, func=AF.Tanh,
                                         bias=vb_sb[:, hc:hc + 1], scale=1.0 / 32)
                    nc.scalar.activation(out=tu[:, hc, :], in_=psu, func=AF.Tanh,
                                         bias=ubh_sb[:, hc:hc + 1], scale=0.5 / 32)
                g = p_act.tile([P, HC, NTOK], BF16, tag="g", name=f"g{k}")
                # A.W = (0.5W).(tv*(tu+1)):  g = (tu + 1) * tv
                nc.vector.scalar_tensor_tensor(out=g, in0=tu, scalar=1.0, in1=tv,
                                               op0=OP.add, op1=OP.mult)
                if STAGE < 1:
                    continue
                ps_s = p_ps.tile([1, NTOK], F32, tag="ps", name=f"pss{k}")
                for hc in range(HC):
                    nc.tensor.matmul(ps_s, w2_sb[:, hc:hc + 1], g[:, hc, :],
                                     start=(hc == 0), stop=(hc == HC - 1))
                e_sb = p_small.tile([1, NTOK], BF16, tag="e", name=f"e{k}")
                nc.scalar.activation(out=e_sb, in_=ps_s, func=AF.Exp,
                                     bias=zero_sb[:, :], scale=1.0)
                em = p_small.tile([1, NTOK], BF16, tag="em", name=f"em{k}")
                nc.vector.scalar_tensor_tensor(out=em, in0=e_sb, scalar=1.0,
                                               in1=mk, op0=OP.mult, op1=OP.mult,
                                               accum_out=den_sb[:, k:k + 1])
                if STAGE < 2:
                    continue
                bc_ps = p_bc.tile([P, NTOK], F32, tag="bc", name=f"bc{k}")
                nc.tensor.matmul(bc_ps, ones_sb, em, start=True, stop=True)
                if STAGE < 3:
                    continue
                scr = p_scr.tile([P, NTOK], BF16, tag="scr", name=f"scr{k}")
                for dc in range(DC):
                    nc.vector.scalar_tensor_tensor(
                        out=scr, in0=xt[:, dc, :], scalar=1.0, in1=bc_ps,
                        op0=OP.mult, op1=OP.mult,
                        accum_out=z_sb[:, k, dc:dc + 1])

            nc.sync.dma_start(out=outz_ext[:, :, :], in_=z_sb)
            nc.gpsimd.dma_start(out=outd_ext[:, :], in_=den_sb)

    nc.finalize()
    return nc


_GRAPHS = {}


def _get_graph(K):
    if K not in _GRAPHS:
        _GRAPHS[K] = build_graph(K)
    return _GRAPHS[K]


def _prep_host(x, lengths, V_w, V_b, U_w, U_b, W_w, W_b):
    lengths = np.maximum(np.asarray(lengths).astype(np.int64), 1)
    groups = np.minimum((lengths + NTOK - 1) // NTOK, NG)
    items = [(b, gi) for b in range(B) for gi in range(int(groups[b]))]
    K = math.ceil(len(items) / NCORES)
    assign = [items[c * K:(c + 1) * K] for c in range(NCORES)]

    def warr(w):  # [D, H] -> [dp, dc, hc, h] fp8, pre-scaled by 32
        return np.ascontiguousarray(
            (w * 32.0).reshape(DC, P, HC, P).transpose(1, 0, 2, 3).astype(NPF8))
    Vw = warr(V_w)
    Uw = warr(U_w)
    Vb = np.ascontiguousarray(V_b.reshape(HC, P).T, dtype=np.float32)
    Ubh = np.ascontiguousarray((U_b * 0.5).reshape(HC, P).T, dtype=np.float32)
    W2 = np.ascontiguousarray(
        (0.5 * W_w[:, 0]).reshape(HC, P).T.astype(ml_dtypes.bfloat16))

    xbf = x.astype(ml_dtypes.bfloat16)  # [B, N, D]
    ar = np.arange(NTOK)

    in_maps = []
    for c in range(NCORES):
        xts = np.zeros((K, P, DC, NTOK), dtype=ml_dtypes.bfloat16)
        msk = np.zeros((K, 1, NTOK), dtype=ml_dtypes.bfloat16)
        for k, (b, gi) in enumerate(assign[c]):
            xg = xbf[b, gi * NTOK:(gi + 1) * NTOK, :]        # [512, 1024]
            xts[k] = xg.reshape(NTOK, DC, P).transpose(2, 1, 0)
            msk[k, 0] = (gi * NTOK + ar < lengths[b])
        in_maps.append({"xT": xts, "xT8": xts.astype(NPF8), "mask": msk,
                        "Vw": Vw, "Uw": Uw,
                        "Vb": Vb, "Ubh": Ubh, "W2": W2})
    return in_maps, assign, K


def kernel(x, lengths, V_w, V_b, U_w, U_b, W_w, W_b, _trace=False, _trace_kwargs=None):
    x = np.asarray(x)
    in_maps, assign, K = _prep_host(
        x, lengths, np.asarray(V_w), np.asarray(V_b), np.asarray(U_w),
        np.asarray(U_b), np.asarray(W_w), np.asarray(W_b),
    )
    nc = _get_graph(K)
    res = run_bass_kernel_spmd(
        nc, in_maps, core_ids=list(range(NCORES)),
        trace=_trace, **(_trace_kwargs or {}),
    )
    z = np.zeros((B, D), dtype=np.float64)
    den = np.zeros((B,), dtype=np.float64)
    for c in range(NCORES):
        zc = np.asarray(res.results[c]["out_z"], dtype=np.float64)   # [P, K, DC]
        dc_ = np.asarray(res.results[c]["out_den"], dtype=np.float64)  # [1, K]
        for k, (b, gi) in enumerate(assign[c]):
            z[b] += zc[:, k, :].T.reshape(D)   # d = dc*128 + p
            den[b] += dc_[0, k]
    out = (z / den[:, None]).astype(np.float32)
    if _trace:
        return out, res
    return out


if __name__ == "__main__":
    rng = np.random.default_rng(0)
    x = rng.standard_normal((B, N, D), dtype=np.float32)
    lengths = rng.integers(0, N, (B,)).astype(np.int32)
    s = 1.0 / np.sqrt(D)
    inputs = dict(
        x=x, lengths=lengths,
        V_w=(rng.standard_normal((D, H), dtype=np.float32) * s),
        V_b=np.zeros(H, np.float32),
        U_w=(rng.standard_normal((D, H), dtype=np.float32) * s),
        U_b=np.zeros(H, np.float32),
        W_w=(rng.standard_normal((H, 1), dtype=np.float32) / 16.0),
        W_b=np.zeros(1, np.float32),
    )
    out = kernel(**inputs)
    print(out.shape, out.dtype)
